# revision 18
# baseline (speedup 1.0000x reference)
"""Chamfer distance kernel for 8 Trainium2 NeuronCores — v16 (final).

TensorE: 4-band row-tiled matmuls (tile_position=(32r,0), K padded 13->32,
operands replicated at partition offsets 0/32/64/96).

Reduction per qtile (8 PSUM groups of 1024 fp32, 4 rotating 2-bank
buffers — the 4-deep rotation gives the matmul refill chain slack so
neither consumer engine ever stalls on PSUM):
  - 6 groups -> ScalarE fp32->bf16 copies; pairs of exits merged by
    VectorE tensor_tensor at 2x bf16, then combined + cascaded to 256
  - 2 groups -> VectorE fused tensor_reduce straight from PSUM (pmw/pm2)
  - batched tail across all 64 qtiles at the end
Input DMA is chunked so the first matmuls start ~7us earlier.
"""

import numpy as np
import ml_dtypes

bf16 = ml_dtypes.bfloat16

B = 4
N = 8192            # points per cloud
NQ = N // 2         # queries per core per pass
K = 13              # real contraction rows (padded to 32 per band)
KP = 32
QT = NQ // 128      # query tiles per pass (32)
NCHUNK = 512        # db points per matmul (one PSUM bank fp32)
GROUP = 2           # chunks per PSUM group tile
GSZ = GROUP * NCHUNK            # 2048
NGROUP = N // GSZ   # 4 groups per qtile
N_CORES = 8
TREE_OUT = 256
DCH = 2048          # input DMA chunk width


def build_bass():
    import concourse.bacc as bacc
    import concourse.mybir as mybir
    from concourse.tile import TileContext

    fp32 = mybir.dt.float32
    bfl6 = mybir.dt.bfloat16
    A = mybir.AluOpType
    AX = mybir.AxisListType
    ACTF = mybir.ActivationFunctionType

    nc = bacc.Bacc()

    la = nc.declare_dram_parameter("la", [128, NQ], bfl6, isOutput=False)
    ra = nc.declare_dram_parameter("ra", [128, N], bfl6, isOutput=False)
    lb = nc.declare_dram_parameter("lb", [128, NQ], bfl6, isOutput=False)
    rb = nc.declare_dram_parameter("rb", [128, N], bfl6, isOutput=False)
    out = nc.declare_dram_parameter("out", [128, 1], fp32, isOutput=True)

    NQT = 2 * QT

    with TileContext(nc) as tc:
        with (
            tc.tile_pool(name="ops", bufs=1) as ops_pool,
            tc.tile_pool(name="psum", bufs=4, space="PSUM") as psum_pool,
            tc.tile_pool(name="exit", bufs=9) as exit_pool,
            tc.tile_pool(name="mrg", bufs=10) as mrg_pool,
        ):
            L = [ops_pool.tile([128, NQ], bfl6, tag="L0", name="L0"),
                 ops_pool.tile([128, NQ], bfl6, tag="L1", name="L1")]
            R = [ops_pool.tile([128, N], bfl6, tag="R0", name="R0"),
                 ops_pool.tile([128, N], bfl6, tag="R1", name="R1")]
            pmw = ops_pool.tile([128, NQT], fp32, tag="pmw")
            pm2 = ops_pool.tile([128, NQT], fp32, tag="pm2")
            W = ops_pool.tile([128, NQT * TREE_OUT], bfl6, tag="W")
            qmin = ops_pool.tile([128, NQT], fp32, tag="qmin")
            acc = ops_pool.tile([128, 1], fp32, tag="acc")

            # chunked input loads: first matmuls only wait on the first
            # chunks of L0/R0 instead of the full 6MB
            srcs = [la, ra, lb, rb]
            dsts = [L[0], R[0], L[1], R[1]]
            fine = [(0, 0, 256), (1, 0, 512), (1, 512, 1024),
                    (0, 256, 512), (1, 1024, 2048), (0, 512, 2048)]
            for ti, lo, hi in fine:
                nc.sync.dma_start(out=dsts[ti][:, lo:hi],
                                  in_=srcs[ti][:, lo:hi])
            order = [(1, 1), (1, 2), (1, 3),
                     (0, 1), (2, 0), (2, 1),
                     (3, 0), (3, 1), (3, 2), (3, 3)]
            for ti, c in order:
                w = srcs[ti].shape[1]
                lo, hi = c * DCH, min((c + 1) * DCH, w)
                if lo >= w:
                    continue
                nc.sync.dma_start(out=dsts[ti][:, lo:hi],
                                  in_=srcs[ti][:, lo:hi])
            for bp in (0, 32, 64, 96):
                nc.tensor.ldweights(L[0][bp:bp + KP, 0:128],
                                    tile_position=(bp, 0))
                nc.tensor.ldweights(R[0][bp:bp + KP, 0:128],
                                    tile_position=(bp, 0))
                nc.tensor.ldweights(L[1][bp:bp + KP, 0:128],
                                    tile_position=(bp, 0))
                nc.tensor.ldweights(R[1][bp:bp + KP, 0:128],
                                    tile_position=(bp, 0))

            H = GSZ // 2
            for p in range(2):
                for t in range(QT):
                    qi = p * QT + t
                    cs = []

                    def mm_group(g):
                        pg = psum_pool.tile([128, GSZ], fp32, tag="pg")
                        for band in range(GROUP):
                            k = g * GROUP + band
                            bp = 32 * (k % 4)
                            j = k * NCHUNK
                            nc.tensor.matmul(
                                pg[:, band * NCHUNK:(band + 1) * NCHUNK],
                                L[p][bp:bp + KP, t * 128:(t + 1) * 128],
                                R[p][bp:bp + KP, j:j + NCHUNK],
                                start=True, stop=True,
                                tile_position=(bp, 0),
                            )
                        return pg

                    def scopy(pg):
                        e = exit_pool.tile([128, GSZ + 64], bfl6, tag="e")
                        nc.scalar.activation(e[:, 0:GSZ], pg[:, :], ACTF.Copy)
                        return e

                    def mfold(ex, ey):
                        c = mrg_pool.tile([128, GSZ + 64], bfl6, tag="c")
                        nc.vector.tensor_tensor(out=c[:, 0:GSZ],
                                                in0=ex[:, 0:GSZ],
                                                in1=ey[:, 0:GSZ], op=A.min)
                        cs.append(c)

                    e0 = scopy(mm_group(0))
                    e1 = scopy(mm_group(1))
                    e2 = scopy(mm_group(2))
                    mfold(e0, e1)
                    e3 = scopy(mm_group(3))
                    e4 = scopy(mm_group(4))
                    mfold(e2, e3)
                    # u1 is always-ready (V-local deps): absorbs red stalls
                    u = mrg_pool.tile([128, 1024 + 64], bfl6, tag="u")
                    nc.vector.tensor_tensor(out=u[:, 0:1024],
                                            in0=cs[0][:, 0:GSZ],
                                            in1=cs[1][:, 0:GSZ], op=A.min)
                    pg6 = mm_group(6)
                    nc.vector.tensor_reduce(
                        out=pmw[:, qi:qi + 1],
                        in_=pg6[:, :], axis=AX.X, op=A.min,
                    )
                    e5 = scopy(mm_group(5))
                    pg7 = mm_group(7)
                    nc.vector.tensor_reduce(
                        out=pm2[:, qi:qi + 1],
                        in_=pg7[:, :], axis=AX.X, op=A.min,
                    )
                    mfold(e4, e5)
                    # combine m45 + cascade to 256
                    nc.vector.tensor_tensor(out=u[:, 0:1024],
                                            in0=u[:, 0:1024],
                                            in1=cs[2][:, 0:GSZ], op=A.min)
                    nc.vector.tensor_tensor(out=u[:, 0:512], in0=u[:, 0:512],
                                            in1=u[:, 512:1024], op=A.min)
                    nc.vector.tensor_tensor(
                        out=W[:, qi * TREE_OUT:(qi + 1) * TREE_OUT],
                        in0=u[:, 0:256], in1=u[:, 256:512], op=A.min)
            # batched tail: all qtiles' [256] blocks -> [1]
            Wv = W.rearrange("p (q n) -> p q n", q=NQT)
            w = TREE_OUT // 2
            while w >= 1:
                nc.vector.tensor_tensor(
                    out=Wv[:, :, 0:w], in0=Wv[:, :, 0:w],
                    in1=Wv[:, :, w:2 * w], op=A.min)
                w //= 2
            # min(direct group, tree) per qtile, clamp, sum
            nc.vector.tensor_tensor(out=qmin[:, :], in0=pmw[:, :],
                                    in1=pm2[:, :], op=A.min)
            nc.vector.tensor_tensor(out=qmin[:, :], in0=qmin[:, :],
                                    in1=Wv[:, :, 0], op=A.min)
            nc.vector.tensor_scalar(out=qmin[:, :], in0=qmin[:, :],
                                    scalar1=0.0, scalar2=None, op0=A.max)
            nc.vector.tensor_reduce(out=acc[:, :], in_=qmin[:, :],
                                    axis=AX.X, op=A.add)
            nc.sync.dma_start(out=out[:, :], in_=acc[:, :])
    nc.finalize()
    return nc


def _split_bf16(x):
    hi = x.astype(bf16)
    lo = (x - hi.astype(np.float32)).astype(bf16)
    return hi, lo


def _pad_bands(rows):
    """[13, n] bf16 -> [128, n]: pad K to 32 with zeros, replicate 4x."""
    n = rows.shape[1]
    k32 = np.zeros((KP, n), dtype=bf16)
    k32[:K] = rows
    return np.concatenate([k32] * 4, axis=0)


def _make_lhsT(q):
    x = np.ascontiguousarray(q.T).astype(np.float32)
    x2 = np.sum(q * q, axis=-1, dtype=np.float32)
    xh, xl = _split_bf16(x)
    x2h, x2l = _split_bf16(x2)
    ones = np.ones_like(x2, dtype=bf16)
    rows = np.concatenate([xh, xh, xl, x2h[None], x2l[None],
                           ones[None], ones[None]], axis=0)
    return _pad_bands(rows)


def _make_rhs(d):
    y = np.ascontiguousarray((-2.0 * d.T)).astype(np.float32)
    y2 = np.sum(d * d, axis=-1, dtype=np.float32)
    yh, yl = _split_bf16(y)
    y2h, y2l = _split_bf16(y2)
    ones = np.ones_like(y2, dtype=bf16)
    rows = np.concatenate([yh, yl, yh, ones[None], ones[None],
                           y2h[None], y2l[None]], axis=0)
    return _pad_bands(rows)


def make_in_maps(points1, points2):
    p1 = np.asarray(points1, dtype=np.float32)
    p2 = np.asarray(points2, dtype=np.float32)
    in_maps = []
    for i in range(N_CORES):
        b, h = divmod(i, 2)
        qa = p1[b, h * NQ:(h + 1) * NQ]
        qb = p2[b, h * NQ:(h + 1) * NQ]
        in_maps.append({
            "la": _make_lhsT(qa), "ra": _make_rhs(p2[b]),
            "lb": _make_lhsT(qb), "rb": _make_rhs(p1[b]),
        })
    return in_maps


_CACHE = {}


def kernel(points1, points2):
    from concourse.bass_utils import run_bass_kernel_spmd

    if "nc" not in _CACHE:
        _CACHE["nc"] = build_bass()
    nc = _CACHE["nc"]
    in_maps = make_in_maps(points1, points2)
    res = run_bass_kernel_spmd(nc, in_maps, core_ids=list(range(N_CORES)))
    total = 0.0
    for i in range(N_CORES):
        total += float(res.results[i]["out"].astype(np.float64).sum())
    return np.float32(total / N)
